# revision 2
# baseline (speedup 1.0000x reference)
"""Trainium2 Bass kernel for CausalSelfAttention with KV cache (decode, T=8).

Sharding: tensor-parallel over heads, 2 heads per core x 8 cores.
Each core: QKV projection for its 2 heads, attention over its KV-cache
shard (fp16), partial row-parallel out-projection (fp32). Host sums the
8 partials and adds b_proj.
"""

import sys

for _p in ("/opt/trn_rl_repo", "/root/.axon_site/_ro/trn_rl_repo"):
    if _p not in sys.path:
        sys.path.insert(0, _p)

import math

import numpy as np

import concourse.bass as bass
import concourse.tile as tile
from concourse import bacc, mybir
from concourse.bass_utils import run_bass_kernel_spmd
from concourse.masks import make_identity

# Problem shape (hardcoded; see spec)
B, T, C = 16, 8, 2048
H, D = 16, 128
MAX_SEQ = 4096
START_POS = 4088
S = MAX_SEQ  # start_pos + T
NCH = S // 128  # 32 S-chunks of 128
BT = B * T  # 128
N_CORES = 8
HPC = H // N_CORES  # heads per core = 2
KC = C // 128  # 16 contraction chunks for projections

F16 = mybir.dt.float16
F32 = mybir.dt.float32

_CACHE = {}


def _build_nc():
    nc = bacc.Bacc("TRN2", target_bir_lowering=False, debug=False)

    kt = nc.dram_tensor("kt", [B, HPC, D, S], F16, kind="ExternalInput").ap()
    vv = nc.dram_tensor("vv", [B, HPC, 128, NCH, 129], F16, kind="ExternalInput").ap()
    wqk = nc.dram_tensor("wqk", [128, KC, 512], F16, kind="ExternalInput").ap()
    bqk = nc.dram_tensor("bqk", [128, 4], F32, kind="ExternalInput").ap()
    wv = nc.dram_tensor("wv", [128, KC, 256], F16, kind="ExternalInput").ap()
    bv = nc.dram_tensor("bv", [1, 256], F16, kind="ExternalInput").ap()
    xt = nc.dram_tensor("xt", [128, KC, 128], F16, kind="ExternalInput").ap()
    wp = nc.dram_tensor("wp", [128, HPC, C], F32, kind="ExternalInput").ap()
    out = nc.dram_tensor("out", [BT, C], F32, kind="ExternalOutput").ap()

    with tile.TileContext(nc) as tc:
        _emit(tc, kt, vv, wqk, bqk, wv, bv, xt, wp, out)
    nc.finalize()
    return nc


def _emit(tc, kt, vv, wqk, bqk, wv, bv, xt, wp, out):
    from contextlib import ExitStack

    nc = tc.nc
    scale = 1.0 / math.sqrt(D)

    with ExitStack() as ctx:
        const = ctx.enter_context(tc.tile_pool(name="const", bufs=1))
        kv = ctx.enter_context(tc.tile_pool(name="kv", bufs=4))
        ptp = ctx.enter_context(tc.tile_pool(name="ptp", bufs=3))
        small = ctx.enter_context(tc.tile_pool(name="small", bufs=3))
        outsb = ctx.enter_context(tc.tile_pool(name="outsb", bufs=2))
        ps_big = ctx.enter_context(tc.tile_pool(name="ps_big", bufs=3, space="PSUM"))
        ps_ya = ctx.enter_context(tc.tile_pool(name="ps_ya", bufs=2, space="PSUM"))
        ps_yt = ctx.enter_context(tc.tile_pool(name="ps_yt", bufs=2, space="PSUM"))

        # ---- constants ----
        wqk_sb = const.tile([128, KC, 512], F16)
        nc.sync.dma_start(wqk_sb[:], wqk)
        wv_sb = const.tile([128, KC, 256], F16)
        nc.sync.dma_start(wv_sb[:], wv)
        xt_sb = const.tile([128, KC, 128], F16)
        nc.sync.dma_start(xt_sb[:], xt)
        bqk_sb = const.tile([128, 4], F32)
        nc.sync.dma_start(bqk_sb[:], bqk)
        bv_sb = const.tile([1, 256], F16)
        nc.sync.dma_start(bv_sb[:], bv)
        wp_sb = const.tile([128, HPC, C], F32)
        nc.sync.dma_start(wp_sb[:], wp)
        ones1 = const.tile([1, 128], F16)
        nc.vector.memset(ones1[:], 1.0)
        id8 = const.tile([8, 8], F32)
        make_identity(nc, id8[:])

        # ---- phase 1: projections ----
        # qkT[m] = (w_qk[:, m-block]).T @ x.T   -> [d, b*t], m in {q0,q1,k0,k1}
        qkT_sb = const.tile([128, 4 * 128], F16)
        for m in range(4):
            ps = ps_big.tile([128, 512], F32, tag="ps_big")
            for c in range(KC):
                nc.tensor.matmul(
                    ps[:, 0:128],
                    lhsT=wqk_sb[:, c, m * 128 : (m + 1) * 128],
                    rhs=xt_sb[:, c, :],
                    start=(c == 0),
                    stop=(c == KC - 1),
                )
            nc.scalar.activation(
                qkT_sb[:, m * 128 : (m + 1) * 128],
                ps[:, 0:128],
                func=mybir.ActivationFunctionType.Identity,
                bias=bqk_sb[:, m : m + 1],
                scale=1.0,
            )

        # v_new = x @ w_v + b_v  -> [b*t, hl*128+d]
        vproj_sb = const.tile([128, 256], F16)
        psv = ps_big.tile([128, 512], F32, tag="ps_big")
        for c in range(KC):
            nc.tensor.matmul(
                psv[:, 0:256],
                lhsT=xt_sb[:, c, :],
                rhs=wv_sb[:, c, :],
                start=(c == 0),
                stop=False,
            )
        nc.tensor.matmul(
            psv[:, 0:256], lhsT=ones1[:], rhs=bv_sb[:], start=False, stop=True
        )
        nc.scalar.copy(vproj_sb[:], psv[:, 0:256])

        # ---- phase 2: attention over 32 (b, hl) pairs ----
        yall_sb = const.tile([128, HPC * 128], F32)
        for b in range(B):
            for hl in range(HPC):
                kt_t = kv.tile([128, S], F16, tag="kt")
                nc.sync.dma_start(kt_t[:, 0:2048], kt[b, hl, :, 0:2048])
                nc.sync.dma_start(kt_t[:, 2048:4096], kt[b, hl, :, 2048:4096])
                # overwrite the 8 new k positions (cols start_pos..start_pos+8)
                nc.vector.tensor_copy(
                    kt_t[:, START_POS : START_POS + 8],
                    qkT_sb[:, (2 + hl) * 128 + b * 8 : (2 + hl) * 128 + b * 8 + 8],
                )
                v_t = kv.tile([128, NCH, 129], F16, tag="v")
                nc.sync.dma_start(v_t[:, 0:16, :], vv[b, hl, :, 0:16, :])
                nc.sync.dma_start(v_t[:, 16:32, :], vv[b, hl, :, 16:32, :])
                # overwrite the 8 new v rows (chunk 31, partitions 120..128)
                nc.sync.dma_start(
                    v_t[120:128, NCH - 1, 0:128],
                    vproj_sb[b * 8 : (b + 1) * 8, hl * 128 : (hl + 1) * 128],
                )

                qT_b = qkT_sb[:, hl * 128 + b * 8 : hl * 128 + b * 8 + 8]

                # scoresT: [s % 128, chunk*8 + t]
                ps_s = ps_big.tile([128, 512], F32, tag="ps_big")
                for c in range(NCH):
                    nc.tensor.matmul(
                        ps_s[:, c * 8 : (c + 1) * 8],
                        lhsT=kt_t[:, c * 128 : (c + 1) * 128],
                        rhs=qT_b,
                        start=(c == 0),
                        stop=(c == NCH - 1),
                    )
                pt_sb = ptp.tile([128, 256], F16)
                nc.scalar.activation(
                    pt_sb[:],
                    ps_s[:, 0:256],
                    func=mybir.ActivationFunctionType.Exp,
                    scale=scale,
                )
                # y_aug = p.T @ [v | 1]: [t, d] plus col 128 = sum(exp)
                ya = ps_ya.tile([8, 129], F32)
                for c in range(NCH):
                    nc.tensor.matmul(
                        ya[:],
                        lhsT=pt_sb[:, c * 8 : (c + 1) * 8],
                        rhs=v_t[:, c, :],
                        start=(c == 0),
                        stop=(c == NCH - 1),
                    )
                rec = small.tile([8, 1], F32, tag="rec")
                nc.vector.reciprocal(rec[:], ya[:, 128:129])
                yn = small.tile([8, 128], F32, tag="yn")
                nc.vector.tensor_scalar_mul(yn[:], ya[:, 0:128], rec[:])
                # transpose to [d, t] and park in yall
                yt = ps_yt.tile([128, 8], F32)
                nc.tensor.transpose(yt[:], yn[:], id8[:])
                nc.vector.tensor_copy(
                    yall_sb[:, hl * 128 + b * 8 : hl * 128 + b * 8 + 8], yt[:]
                )

        # ---- phase 3: partial out-projection (fp32) ----
        for nb in range(4):
            pso = ps_big.tile([128, 512], F32, tag="ps_big")
            for kc in range(HPC):
                nc.tensor.matmul(
                    pso[:],
                    lhsT=yall_sb[:, kc * 128 : (kc + 1) * 128],
                    rhs=wp_sb[:, kc, nb * 512 : (nb + 1) * 512],
                    start=(kc == 0),
                    stop=(kc == HPC - 1),
                )
            osb = outsb.tile([128, 512], F32)
            nc.vector.tensor_copy(osb[:], pso[:])
            nc.sync.dma_start(out[:, nb * 512 : (nb + 1) * 512], osb[:])


def _prep_core_inputs(core, x2d, k_cache, v_cache, w_attn, b_attn, w_proj):
    hg0 = HPC * core
    f16 = np.float16

    kc = k_cache[:, hg0 : hg0 + HPC].astype(f16)  # [B, HPC, S, D]
    kt = np.ascontiguousarray(kc.transpose(0, 1, 3, 2))  # [B, HPC, D, S]

    vc = v_cache[:, hg0 : hg0 + HPC].astype(f16)  # [B, HPC, S, D]
    vv = np.empty((B, HPC, 128, NCH, 129), f16)
    vv[..., :128] = vc.reshape(B, HPC, NCH, 128, D).transpose(0, 1, 3, 2, 4)
    vv[..., 128] = 1.0

    # wqk[p, c, m*128+j]: m in {q_h0, q_h1, k_h0, k_h1}
    cols = []
    for m in range(2):  # q block then k block
        for hl in range(HPC):
            base = m * C + (hg0 + hl) * D
            cols.append(np.arange(base, base + D))
    cols = np.concatenate(cols)  # [512]
    wqk = np.ascontiguousarray(
        w_attn[:, cols].reshape(KC, 128, 512).transpose(1, 0, 2)
    ).astype(f16)
    bqk = np.ascontiguousarray(b_attn[cols].reshape(4, 128).T).astype(np.float32)

    vcols = np.arange(2 * C + hg0 * D, 2 * C + (hg0 + HPC) * D)  # [256]
    wv = np.ascontiguousarray(
        w_attn[:, vcols].reshape(KC, 128, 256).transpose(1, 0, 2)
    ).astype(f16)
    bv = b_attn[vcols].reshape(1, 256).astype(f16)

    xt = np.ascontiguousarray(x2d.T.reshape(KC, 128, 128).transpose(1, 0, 2)).astype(
        f16
    )

    wpl = w_proj[hg0 * D : (hg0 + HPC) * D, :]  # [256, C]
    wp = np.ascontiguousarray(wpl.reshape(HPC, 128, C).transpose(1, 0, 2)).astype(
        np.float32
    )

    return {
        "kt": kt,
        "vv": vv,
        "wqk": wqk,
        "bqk": bqk,
        "wv": wv,
        "bv": bv,
        "xt": xt,
        "wp": wp,
    }


def kernel(
    x,
    k_cache,
    v_cache,
    w_attn,
    b_attn,
    w_proj,
    b_proj,
    start_pos,
    is_causal,
):
    x = np.asarray(x, dtype=np.float32)
    k_cache = np.asarray(k_cache, dtype=np.float32)
    v_cache = np.asarray(v_cache, dtype=np.float32)
    w_attn = np.asarray(w_attn, dtype=np.float32)
    b_attn = np.asarray(b_attn, dtype=np.float32)
    w_proj = np.asarray(w_proj, dtype=np.float32)
    b_proj = np.asarray(b_proj, dtype=np.float32)
    assert int(start_pos) == START_POS, f"kernel hardcodes start_pos={START_POS}"
    assert int(is_causal) == 0, "kernel hardcodes is_causal=0"

    if "nc" not in _CACHE:
        _CACHE["nc"] = _build_nc()
    nc = _CACHE["nc"]

    x2d = x.reshape(BT, C)
    in_maps = [
        _prep_core_inputs(c, x2d, k_cache, v_cache, w_attn, b_attn, w_proj)
        for c in range(N_CORES)
    ]
    res = run_bass_kernel_spmd(nc, in_maps, core_ids=list(range(N_CORES)))
    acc = np.zeros((BT, C), np.float64)
    for c in range(N_CORES):
        acc += res.results[c]["out"]
    y = (acc + b_proj).astype(np.float32)
    return y.reshape(B, T, C)


if __name__ == "__main__":
    # quick self-run against the local reference
    sys.path.insert(0, "/root/problem")
    import reference

    inputs = {k: np.asarray(v) for k, v in reference.setup_inputs().items()}
    expected = np.asarray(reference.reference(**reference.setup_inputs()))
    actual = kernel(**inputs)
    err = np.abs(actual - expected)
    rel = err.max() / np.abs(expected).max()
    print("max abs err:", err.max(), "rel:", rel)


# revision 7
# speedup vs baseline: 1.2343x; 1.2343x over previous
"""Trainium2 Bass kernel for CausalSelfAttention with KV cache (decode, T=8).

Sharding: tensor-parallel over heads, 2 heads per core x 8 cores.
Each core: QKV projection for its 2 heads, attention over its KV-cache
shard (fp16), partial row-parallel out-projection (fp32). Host sums the
8 partials and adds b_proj.
"""

import sys

for _p in ("/opt/trn_rl_repo", "/root/.axon_site/_ro/trn_rl_repo"):
    if _p not in sys.path:
        sys.path.insert(0, _p)

import math

import numpy as np

import concourse.bass as bass
import concourse.tile as tile
from concourse import bacc, mybir
from concourse.bass_utils import run_bass_kernel_spmd
from concourse.masks import make_identity

# Problem shape (hardcoded; see spec)
B, T, C = 16, 8, 2048
H, D = 16, 128
MAX_SEQ = 4096
START_POS = 4088
S = MAX_SEQ  # start_pos + T
NCH = S // 128  # 32 S-chunks of 128
BT = B * T  # 128
N_CORES = 8
HPC = H // N_CORES  # heads per core = 2
KC = C // 128  # 16 contraction chunks for projections

F16 = mybir.dt.float16
F32 = mybir.dt.float32

_CACHE = {}


def _build_nc(repeat=1):
    nc = bacc.Bacc("TRN2", target_bir_lowering=False, debug=False)

    kt = nc.dram_tensor("kt", [B, HPC, D, S], F16, kind="ExternalInput").ap()
    vv = nc.dram_tensor("vv", [B, HPC, 128, NCH, 129], F16, kind="ExternalInput").ap()
    wqk = nc.dram_tensor("wqk", [128, KC, 512], F16, kind="ExternalInput").ap()
    bqk = nc.dram_tensor("bqk", [128, 4], F32, kind="ExternalInput").ap()
    wv = nc.dram_tensor("wv", [128, KC, 256], F16, kind="ExternalInput").ap()
    bv = nc.dram_tensor("bv", [1, 256], F16, kind="ExternalInput").ap()
    xt = nc.dram_tensor("xt", [128, KC, 128], F16, kind="ExternalInput").ap()
    wp = nc.dram_tensor("wp", [128, HPC, C], F16, kind="ExternalInput").ap()
    out = nc.dram_tensor("out", [BT, C], F32, kind="ExternalOutput").ap()

    with tile.TileContext(nc) as tc:
        for _ in range(repeat):
            _emit(tc, kt, vv, wqk, bqk, wv, bv, xt, wp, out)
    nc.finalize()
    return nc


def _emit(tc, kt, vv, wqk, bqk, wv, bv, xt, wp, out):
    from contextlib import ExitStack

    nc = tc.nc
    scale = 1.0 / math.sqrt(D)

    with ExitStack() as ctx:
        const = ctx.enter_context(tc.tile_pool(name="const", bufs=1))
        kv = ctx.enter_context(tc.tile_pool(name="kv", bufs=5))
        ptp = ctx.enter_context(tc.tile_pool(name="ptp", bufs=3))
        small = ctx.enter_context(tc.tile_pool(name="small", bufs=3))
        outsb = ctx.enter_context(tc.tile_pool(name="outsb", bufs=2))
        ps_big = ctx.enter_context(tc.tile_pool(name="ps_big", bufs=3, space="PSUM"))
        ps_ya = ctx.enter_context(tc.tile_pool(name="ps_ya", bufs=2, space="PSUM"))
        ps_yt = ctx.enter_context(tc.tile_pool(name="ps_yt", bufs=2, space="PSUM"))

        # ---- constants ----
        wqk_sb = const.tile([128, KC, 512], F16)
        nc.sync.dma_start(wqk_sb[:], wqk)
        wv_sb = const.tile([128, KC, 256], F16)
        nc.sync.dma_start(wv_sb[:], wv)
        xt_sb = const.tile([128, KC, 128], F16)
        nc.sync.dma_start(xt_sb[:], xt)
        bqk_sb = const.tile([128, 4], F32)
        nc.sync.dma_start(bqk_sb[:], bqk)
        bv_sb = const.tile([1, 256], F16)
        nc.sync.dma_start(bv_sb[:], bv)
        wp_sb = const.tile([128, HPC, C], F16)
        nc.sync.dma_start(wp_sb[:], wp)
        ones1 = const.tile([1, 128], F16)
        nc.vector.memset(ones1[:], 1.0)
        id8 = const.tile([8, 8], F32)
        make_identity(nc, id8[:])

        # ---- phase 1: projections ----
        # qkT[m] = (w_qk[:, m-block]).T @ x.T   -> [d, b*t], m in {q0,q1,k0,k1}
        qkT_sb = const.tile([128, 4 * 128], F16)
        for m in range(4):
            ps = ps_big.tile([128, 512], F32, tag="ps_big")
            for c in range(KC):
                nc.tensor.matmul(
                    ps[:, 0:128],
                    lhsT=wqk_sb[:, c, m * 128 : (m + 1) * 128],
                    rhs=xt_sb[:, c, :],
                    start=(c == 0),
                    stop=(c == KC - 1),
                )
            nc.scalar.activation(
                qkT_sb[:, m * 128 : (m + 1) * 128],
                ps[:, 0:128],
                func=mybir.ActivationFunctionType.Identity,
                bias=bqk_sb[:, m : m + 1],
                scale=1.0,
            )

        # v_new = x @ w_v + b_v  -> [b*t, hl*128+d]
        vproj_sb = const.tile([128, 256], F16)
        psv = ps_big.tile([128, 512], F32, tag="ps_big")
        for c in range(KC):
            nc.tensor.matmul(
                psv[:, 0:256],
                lhsT=xt_sb[:, c, :],
                rhs=wv_sb[:, c, :],
                start=(c == 0),
                stop=False,
            )
        nc.tensor.matmul(
            psv[:, 0:256], lhsT=ones1[:], rhs=bv_sb[:], start=False, stop=True
        )
        nc.scalar.copy(vproj_sb[:], psv[:, 0:256])

        # ---- phase 2: attention over 32 (b, hl) pairs ----
        yall_sb = const.tile([128, HPC * 128], F16)
        for b in range(B):
            for hl in range(HPC):
                kt_t = kv.tile([128, S], F16, tag="kt")
                for q in range(4):
                    nc.sync.dma_start(
                        kt_t[:, q * 1024 : (q + 1) * 1024],
                        kt[b, hl, :, q * 1024 : (q + 1) * 1024],
                    )
                # overwrite the 8 new k positions (cols start_pos..start_pos+8)
                nc.vector.tensor_copy(
                    kt_t[:, START_POS : START_POS + 8],
                    qkT_sb[:, (2 + hl) * 128 + b * 8 : (2 + hl) * 128 + b * 8 + 8],
                )
                v_t = kv.tile([128, NCH, 129], F16, tag="v")
                for q in range(4):
                    nc.sync.dma_start(
                        v_t[:, q * 8 : (q + 1) * 8, :], vv[b, hl, :, q * 8 : (q + 1) * 8, :]
                    )
                # overwrite the 8 new v rows (chunk 31, partitions 120..128)
                nc.gpsimd.dma_start(
                    v_t[120:128, NCH - 1, 0:128],
                    vproj_sb[b * 8 : (b + 1) * 8, hl * 128 : (hl + 1) * 128],
                )

                qT_b = qkT_sb[:, hl * 128 + b * 8 : hl * 128 + b * 8 + 8]

                # scoresT: [s % 128, chunk*8 + t]
                ps_s = ps_big.tile([128, 512], F32, tag="ps_big")
                for c in range(NCH):
                    nc.tensor.matmul(
                        ps_s[:, c * 8 : (c + 1) * 8],
                        lhsT=kt_t[:, c * 128 : (c + 1) * 128],
                        rhs=qT_b,
                        start=(c == 0),
                        stop=(c == NCH - 1),
                    )
                pt_sb = ptp.tile([128, 256], F16)
                nc.scalar.activation(
                    pt_sb[:],
                    ps_s[:, 0:256],
                    func=mybir.ActivationFunctionType.Exp,
                    scale=scale,
                )
                # y_aug = p.T @ [v | 1]: [t, d] plus col 128 = sum(exp)
                ya = ps_ya.tile([8, 129], F32)
                for c in range(NCH):
                    nc.tensor.matmul(
                        ya[:],
                        lhsT=pt_sb[:, c * 8 : (c + 1) * 8],
                        rhs=v_t[:, c, :],
                        start=(c == 0),
                        stop=(c == NCH - 1),
                    )
                rec = small.tile([8, 1], F32, tag="rec")
                nc.vector.reciprocal(rec[:], ya[:, 128:129])
                yn = small.tile([8, 128], F32, tag="yn")
                nc.vector.tensor_scalar_mul(yn[:], ya[:, 0:128], rec[:])
                # transpose to [d, t] and park in yall
                yt = ps_yt.tile([128, 8], F32)
                nc.tensor.transpose(yt[:], yn[:], id8[:])
                nc.vector.tensor_copy(
                    yall_sb[:, hl * 128 + b * 8 : hl * 128 + b * 8 + 8], yt[:]
                )

        # ---- phase 3: partial out-projection (fp32) ----
        for nb in range(4):
            pso = ps_big.tile([128, 512], F32, tag="ps_big")
            for kc in range(HPC):
                nc.tensor.matmul(
                    pso[:],
                    lhsT=yall_sb[:, kc * 128 : (kc + 1) * 128],
                    rhs=wp_sb[:, kc, nb * 512 : (nb + 1) * 512],
                    start=(kc == 0),
                    stop=(kc == HPC - 1),
                )
            osb = outsb.tile([128, 512], F32)
            nc.vector.tensor_copy(osb[:], pso[:])
            nc.sync.dma_start(out[:, nb * 512 : (nb + 1) * 512], osb[:])


def _prep_core_inputs(core, x2d, k_cache, v_cache, w_attn, b_attn, w_proj):
    hg0 = HPC * core
    f16 = np.float16

    kc = k_cache[:, hg0 : hg0 + HPC].astype(f16)  # [B, HPC, S, D]
    kt = np.ascontiguousarray(kc.transpose(0, 1, 3, 2))  # [B, HPC, D, S]

    vc = v_cache[:, hg0 : hg0 + HPC].astype(f16)  # [B, HPC, S, D]
    vv = np.empty((B, HPC, 128, NCH, 129), f16)
    vv[..., :128] = vc.reshape(B, HPC, NCH, 128, D).transpose(0, 1, 3, 2, 4)
    vv[..., 128] = 1.0

    # wqk[p, c, m*128+j]: m in {q_h0, q_h1, k_h0, k_h1}
    cols = []
    for m in range(2):  # q block then k block
        for hl in range(HPC):
            base = m * C + (hg0 + hl) * D
            cols.append(np.arange(base, base + D))
    cols = np.concatenate(cols)  # [512]
    wqk = np.ascontiguousarray(
        w_attn[:, cols].reshape(KC, 128, 512).transpose(1, 0, 2)
    ).astype(f16)
    bqk = np.ascontiguousarray(b_attn[cols].reshape(4, 128).T).astype(np.float32)

    vcols = np.arange(2 * C + hg0 * D, 2 * C + (hg0 + HPC) * D)  # [256]
    wv = np.ascontiguousarray(
        w_attn[:, vcols].reshape(KC, 128, 256).transpose(1, 0, 2)
    ).astype(f16)
    bv = b_attn[vcols].reshape(1, 256).astype(f16)

    xt = np.ascontiguousarray(x2d.T.reshape(KC, 128, 128).transpose(1, 0, 2)).astype(
        f16
    )

    wpl = w_proj[hg0 * D : (hg0 + HPC) * D, :]  # [256, C]
    wp = np.ascontiguousarray(wpl.reshape(HPC, 128, C).transpose(1, 0, 2)).astype(
        np.float16
    )

    return {
        "kt": kt,
        "vv": vv,
        "wqk": wqk,
        "bqk": bqk,
        "wv": wv,
        "bv": bv,
        "xt": xt,
        "wp": wp,
    }


def kernel(
    x,
    k_cache,
    v_cache,
    w_attn,
    b_attn,
    w_proj,
    b_proj,
    start_pos,
    is_causal,
):
    x = np.asarray(x, dtype=np.float32)
    k_cache = np.asarray(k_cache, dtype=np.float32)
    v_cache = np.asarray(v_cache, dtype=np.float32)
    w_attn = np.asarray(w_attn, dtype=np.float32)
    b_attn = np.asarray(b_attn, dtype=np.float32)
    w_proj = np.asarray(w_proj, dtype=np.float32)
    b_proj = np.asarray(b_proj, dtype=np.float32)
    assert int(start_pos) == START_POS, f"kernel hardcodes start_pos={START_POS}"
    assert int(is_causal) == 0, "kernel hardcodes is_causal=0"

    if "nc" not in _CACHE:
        _CACHE["nc"] = _build_nc()
    nc = _CACHE["nc"]

    x2d = x.reshape(BT, C)
    in_maps = [
        _prep_core_inputs(c, x2d, k_cache, v_cache, w_attn, b_attn, w_proj)
        for c in range(N_CORES)
    ]
    res = run_bass_kernel_spmd(nc, in_maps, core_ids=list(range(N_CORES)))
    acc = np.zeros((BT, C), np.float64)
    for c in range(N_CORES):
        acc += res.results[c]["out"]
    y = (acc + b_proj).astype(np.float32)
    return y.reshape(B, T, C)


if __name__ == "__main__":
    # quick self-run against the local reference
    sys.path.insert(0, "/root/problem")
    import reference

    inputs = {k: np.asarray(v) for k, v in reference.setup_inputs().items()}
    expected = np.asarray(reference.reference(**reference.setup_inputs()))
    actual = kernel(**inputs)
    err = np.abs(actual - expected)
    rel = err.max() / np.abs(expected).max()
    print("max abs err:", err.max(), "rel:", rel)


# revision 9
# speedup vs baseline: 1.2453x; 1.0089x over previous
"""Trainium2 Bass kernel for CausalSelfAttention with KV cache (decode, T=8).

Sharding: tensor-parallel over heads, 2 heads per core x 8 cores.
Each core: QKV projection for its 2 heads, attention over its KV-cache
shard (fp16), partial row-parallel out-projection (fp32). Host sums the
8 partials and adds b_proj.
"""

import sys

for _p in ("/opt/trn_rl_repo", "/root/.axon_site/_ro/trn_rl_repo"):
    if _p not in sys.path:
        sys.path.insert(0, _p)

import math

import numpy as np

import concourse.bass as bass
import concourse.tile as tile
from concourse import bacc, mybir
from concourse.bass_utils import run_bass_kernel_spmd
from concourse.masks import make_identity

# Problem shape (hardcoded; see spec)
B, T, C = 16, 8, 2048
H, D = 16, 128
MAX_SEQ = 4096
START_POS = 4088
S = MAX_SEQ  # start_pos + T
NCH = S // 128  # 32 S-chunks of 128
BT = B * T  # 128
N_CORES = 8
HPC = H // N_CORES  # heads per core = 2
KC = C // 128  # 16 contraction chunks for projections

F16 = mybir.dt.float16
F32 = mybir.dt.float32

_CACHE = {}


def _build_nc(repeat=1):
    nc = bacc.Bacc("TRN2", target_bir_lowering=False, debug=False)

    kt = nc.dram_tensor("kt", [B, HPC, D, S], F16, kind="ExternalInput").ap()
    vv = nc.dram_tensor("vv", [B, HPC, 128, NCH, 128], F16, kind="ExternalInput").ap()
    wqk = nc.dram_tensor("wqk", [128, KC, 512], F16, kind="ExternalInput").ap()
    bqk = nc.dram_tensor("bqk", [128, 4], F32, kind="ExternalInput").ap()
    wv = nc.dram_tensor("wv", [128, KC, 256], F16, kind="ExternalInput").ap()
    bv = nc.dram_tensor("bv", [1, 256], F16, kind="ExternalInput").ap()
    xt = nc.dram_tensor("xt", [128, KC, 128], F16, kind="ExternalInput").ap()
    wp = nc.dram_tensor("wp", [128, HPC, C], F16, kind="ExternalInput").ap()
    out = nc.dram_tensor("out", [BT, C], F16, kind="ExternalOutput").ap()

    with tile.TileContext(nc) as tc:
        for _ in range(repeat):
            _emit(tc, kt, vv, wqk, bqk, wv, bv, xt, wp, out)
    nc.finalize()
    return nc


def _emit(tc, kt, vv, wqk, bqk, wv, bv, xt, wp, out):
    from contextlib import ExitStack

    nc = tc.nc
    scale = 1.0 / math.sqrt(D)

    with ExitStack() as ctx:
        const = ctx.enter_context(tc.tile_pool(name="const", bufs=1))
        kv = ctx.enter_context(tc.tile_pool(name="kv", bufs=5))
        ptp = ctx.enter_context(tc.tile_pool(name="ptp", bufs=3))
        small = ctx.enter_context(tc.tile_pool(name="small", bufs=3))
        outsb = ctx.enter_context(tc.tile_pool(name="outsb", bufs=2))
        ps_big = ctx.enter_context(tc.tile_pool(name="ps_big", bufs=3, space="PSUM"))
        ps_ya = ctx.enter_context(tc.tile_pool(name="ps_ya", bufs=2, space="PSUM"))
        ps_yt = ctx.enter_context(tc.tile_pool(name="ps_yt", bufs=2, space="PSUM"))
        ps_sums = ctx.enter_context(tc.tile_pool(name="ps_sums", bufs=1, space="PSUM"))

        # ---- constants ----
        wqk_sb = const.tile([128, KC, 512], F16)
        nc.sync.dma_start(wqk_sb[:], wqk)
        wv_sb = const.tile([128, KC, 256], F16)
        nc.sync.dma_start(wv_sb[:], wv)
        xt_sb = const.tile([128, KC, 128], F16)
        nc.sync.dma_start(xt_sb[:], xt)
        bqk_sb = const.tile([128, 4], F32)
        nc.sync.dma_start(bqk_sb[:], bqk)
        bv_sb = const.tile([1, 256], F16)
        nc.sync.dma_start(bv_sb[:], bv)
        wp_sb = const.tile([128, HPC, C], F16)
        nc.sync.dma_start(wp_sb[:], wp)
        ones1 = const.tile([1, 128], F16)
        nc.vector.memset(ones1[:], 1.0)
        ones128 = const.tile([128, 1], F32)
        nc.vector.memset(ones128[:], 1.0)
        id8 = const.tile([8, 8], F32)
        make_identity(nc, id8[:])

        # ---- phase 1: projections ----
        # qkT[m] = (w_qk[:, m-block]).T @ x.T   -> [d, b*t], m in {q0,q1,k0,k1}
        qkT_sb = const.tile([128, 4 * 128], F16)
        for m in range(4):
            ps = ps_big.tile([128, 512], F32, tag="ps_big")
            for c in range(KC):
                nc.tensor.matmul(
                    ps[:, 0:128],
                    lhsT=wqk_sb[:, c, m * 128 : (m + 1) * 128],
                    rhs=xt_sb[:, c, :],
                    start=(c == 0),
                    stop=(c == KC - 1),
                )
            nc.scalar.activation(
                qkT_sb[:, m * 128 : (m + 1) * 128],
                ps[:, 0:128],
                func=mybir.ActivationFunctionType.Identity,
                bias=bqk_sb[:, m : m + 1],
                scale=1.0,
            )

        # v_new = x @ w_v + b_v  -> [b*t, hl*128+d]
        vproj_sb = const.tile([128, 256], F16)
        psv = ps_big.tile([128, 512], F32, tag="ps_big")
        for c in range(KC):
            nc.tensor.matmul(
                psv[:, 0:256],
                lhsT=xt_sb[:, c, :],
                rhs=wv_sb[:, c, :],
                start=(c == 0),
                stop=False,
            )
        nc.tensor.matmul(
            psv[:, 0:256], lhsT=ones1[:], rhs=bv_sb[:], start=False, stop=True
        )
        nc.scalar.copy(vproj_sb[:], psv[:, 0:256])

        # ---- phase 2: attention over 32 (b, hl) pairs ----
        yall_sb = const.tile([128, HPC * 128], F16)
        for b in range(B):
            for hl in range(HPC):
                kt_t = kv.tile([128, S], F16, tag="kt")
                for q in range(4):
                    hi = min((q + 1) * 1024, START_POS)
                    nc.sync.dma_start(
                        kt_t[:, q * 1024 : hi],
                        kt[b, hl, :, q * 1024 : hi],
                    )
                # overwrite the 8 new k positions (cols start_pos..start_pos+8)
                nc.vector.tensor_copy(
                    kt_t[:, START_POS : START_POS + 8],
                    qkT_sb[:, (2 + hl) * 128 + b * 8 : (2 + hl) * 128 + b * 8 + 8],
                )
                v_t = kv.tile([128, NCH, 128], F16, tag="v")
                for q in range(4):
                    nc.sync.dma_start(
                        v_t[:, q * 8 : (q + 1) * 8, :], vv[b, hl, :, q * 8 : (q + 1) * 8, :]
                    )
                # overwrite the 8 new v rows (chunk 31, partitions 120..128)
                nc.gpsimd.dma_start(
                    v_t[120:128, NCH - 1, 0:128],
                    vproj_sb[b * 8 : (b + 1) * 8, hl * 128 : (hl + 1) * 128],
                )

                qT_b = qkT_sb[:, hl * 128 + b * 8 : hl * 128 + b * 8 + 8]

                # scoresT: [s % 128, chunk*8 + t]
                ps_s = ps_big.tile([128, 512], F32, tag="ps_big")
                for c in range(NCH):
                    nc.tensor.matmul(
                        ps_s[:, c * 8 : (c + 1) * 8],
                        lhsT=kt_t[:, c * 128 : (c + 1) * 128],
                        rhs=qT_b,
                        start=(c == 0),
                        stop=(c == NCH - 1),
                    )
                pt_sb = ptp.tile([128, 256], F16)
                nc.scalar.activation(
                    pt_sb[:],
                    ps_s[:, 0:256],
                    func=mybir.ActivationFunctionType.Exp,
                    scale=scale,
                )
                # y = p.T @ v : [t, d]
                ya = ps_ya.tile([8, 128], F32)
                for c in range(NCH):
                    nc.tensor.matmul(
                        ya[:],
                        lhsT=pt_sb[:, c * 8 : (c + 1) * 8],
                        rhs=v_t[:, c, :],
                        start=(c == 0),
                        stop=(c == NCH - 1),
                    )
                # softmax sums: reduce p over chunks on DVE, then over s%128 on PE
                csum = small.tile([128, 8], F32, tag="csum")
                nc.vector.reduce_sum(
                    csum[:], pt_sb[:].rearrange("p (c t) -> p t c", t=8),
                    axis=mybir.AxisListType.X,
                )
                sums = ps_sums.tile([8, 1], F32, tag="sums")
                nc.tensor.matmul(sums[:], lhsT=csum[:], rhs=ones128[:])
                rec = small.tile([8, 1], F32, tag="rec")
                nc.vector.reciprocal(rec[:], sums[:])
                yn = small.tile([8, 128], F32, tag="yn")
                nc.vector.tensor_scalar_mul(yn[:], ya[:], rec[:])
                # transpose to [d, t] and park in yall
                yt = ps_yt.tile([128, 8], F32)
                nc.tensor.transpose(yt[:], yn[:], id8[:])
                nc.vector.tensor_copy(
                    yall_sb[:, hl * 128 + b * 8 : hl * 128 + b * 8 + 8], yt[:]
                )

        # ---- phase 3: partial out-projection (fp32) ----
        for nb in range(4):
            pso = ps_big.tile([128, 512], F32, tag="ps_big")
            for kc in range(HPC):
                nc.tensor.matmul(
                    pso[:],
                    lhsT=yall_sb[:, kc * 128 : (kc + 1) * 128],
                    rhs=wp_sb[:, kc, nb * 512 : (nb + 1) * 512],
                    start=(kc == 0),
                    stop=(kc == HPC - 1),
                )
            osb = outsb.tile([128, 512], F16)
            nc.vector.tensor_copy(osb[:], pso[:])
            nc.sync.dma_start(out[:, nb * 512 : (nb + 1) * 512], osb[:])


def _prep_core_inputs(core, x2d, k_cache, v_cache, w_attn, b_attn, w_proj):
    hg0 = HPC * core
    f16 = np.float16

    kc = k_cache[:, hg0 : hg0 + HPC].astype(f16)  # [B, HPC, S, D]
    kt = np.ascontiguousarray(kc.transpose(0, 1, 3, 2))  # [B, HPC, D, S]

    vc = v_cache[:, hg0 : hg0 + HPC].astype(f16)  # [B, HPC, S, D]
    vv = np.ascontiguousarray(
        vc.reshape(B, HPC, NCH, 128, D).transpose(0, 1, 3, 2, 4)
    )

    # wqk[p, c, m*128+j]: m in {q_h0, q_h1, k_h0, k_h1}
    cols = []
    for m in range(2):  # q block then k block
        for hl in range(HPC):
            base = m * C + (hg0 + hl) * D
            cols.append(np.arange(base, base + D))
    cols = np.concatenate(cols)  # [512]
    wqk = np.ascontiguousarray(
        w_attn[:, cols].reshape(KC, 128, 512).transpose(1, 0, 2)
    ).astype(f16)
    bqk = np.ascontiguousarray(b_attn[cols].reshape(4, 128).T).astype(np.float32)

    vcols = np.arange(2 * C + hg0 * D, 2 * C + (hg0 + HPC) * D)  # [256]
    wv = np.ascontiguousarray(
        w_attn[:, vcols].reshape(KC, 128, 256).transpose(1, 0, 2)
    ).astype(f16)
    bv = b_attn[vcols].reshape(1, 256).astype(f16)

    xt = np.ascontiguousarray(x2d.T.reshape(KC, 128, 128).transpose(1, 0, 2)).astype(
        f16
    )

    wpl = w_proj[hg0 * D : (hg0 + HPC) * D, :]  # [256, C]
    wp = np.ascontiguousarray(wpl.reshape(HPC, 128, C).transpose(1, 0, 2)).astype(
        np.float16
    )

    return {
        "kt": kt,
        "vv": vv,
        "wqk": wqk,
        "bqk": bqk,
        "wv": wv,
        "bv": bv,
        "xt": xt,
        "wp": wp,
    }


def kernel(
    x,
    k_cache,
    v_cache,
    w_attn,
    b_attn,
    w_proj,
    b_proj,
    start_pos,
    is_causal,
):
    x = np.asarray(x, dtype=np.float32)
    k_cache = np.asarray(k_cache, dtype=np.float32)
    v_cache = np.asarray(v_cache, dtype=np.float32)
    w_attn = np.asarray(w_attn, dtype=np.float32)
    b_attn = np.asarray(b_attn, dtype=np.float32)
    w_proj = np.asarray(w_proj, dtype=np.float32)
    b_proj = np.asarray(b_proj, dtype=np.float32)
    assert int(start_pos) == START_POS, f"kernel hardcodes start_pos={START_POS}"
    assert int(is_causal) == 0, "kernel hardcodes is_causal=0"

    if "nc" not in _CACHE:
        _CACHE["nc"] = _build_nc()
    nc = _CACHE["nc"]

    x2d = x.reshape(BT, C)
    in_maps = [
        _prep_core_inputs(c, x2d, k_cache, v_cache, w_attn, b_attn, w_proj)
        for c in range(N_CORES)
    ]
    res = run_bass_kernel_spmd(nc, in_maps, core_ids=list(range(N_CORES)))
    acc = np.zeros((BT, C), np.float64)
    for c in range(N_CORES):
        acc += res.results[c]["out"].astype(np.float64)
    y = (acc + b_proj).astype(np.float32)
    return y.reshape(B, T, C)


if __name__ == "__main__":
    # quick self-run against the local reference
    sys.path.insert(0, "/root/problem")
    import reference

    inputs = {k: np.asarray(v) for k, v in reference.setup_inputs().items()}
    expected = np.asarray(reference.reference(**reference.setup_inputs()))
    actual = kernel(**inputs)
    err = np.abs(actual - expected)
    rel = err.max() / np.abs(expected).max()
    print("max abs err:", err.max(), "rel:", rel)


# revision 13
# speedup vs baseline: 30.5399x; 24.5241x over previous
"""Trainium2 Bass kernel for CausalSelfAttention with KV cache (decode, T=8).

Sharding: tensor-parallel over heads, 2 heads per core x 8 cores.
Each core: QKV projection for its 2 heads, attention over its KV-cache
shard (fp16), partial row-parallel out-projection (fp32). Host sums the
8 partials and adds b_proj.
"""

import sys

for _p in ("/opt/trn_rl_repo", "/root/.axon_site/_ro/trn_rl_repo"):
    if _p not in sys.path:
        sys.path.insert(0, _p)

import math

import numpy as np

import concourse.bass as bass
import concourse.tile as tile
from concourse import bacc, mybir
from concourse.masks import make_identity

# Problem shape (hardcoded; see spec)
B, T, C = 16, 8, 2048
H, D = 16, 128
MAX_SEQ = 4096
START_POS = 4088
S = MAX_SEQ  # start_pos + T
NCH = S // 128  # 32 S-chunks of 128
BT = B * T  # 128
N_CORES = 8
HPC = H // N_CORES  # heads per core = 2
KC = C // 128  # 16 contraction chunks for projections

F16 = mybir.dt.float16
F32 = mybir.dt.float32

_CACHE = {}


def _build_nc(repeat=1):
    nc = bacc.Bacc("TRN2", target_bir_lowering=False, debug=False)

    kt = nc.dram_tensor("kt", [B, HPC, D, S], F16, kind="ExternalInput").ap()
    vv = nc.dram_tensor("vv", [B, HPC, 128, NCH, 128], F16, kind="ExternalInput").ap()
    wqk = nc.dram_tensor("wqk", [128, KC, 512], F16, kind="ExternalInput").ap()
    bqk = nc.dram_tensor("bqk", [128, 4], F32, kind="ExternalInput").ap()
    wv = nc.dram_tensor("wv", [128, KC, 256], F16, kind="ExternalInput").ap()
    bv = nc.dram_tensor("bv", [1, 256], F16, kind="ExternalInput").ap()
    xt = nc.dram_tensor("xt", [128, KC, 128], F16, kind="ExternalInput").ap()
    wp = nc.dram_tensor("wp", [128, HPC, C], F16, kind="ExternalInput").ap()
    out = nc.dram_tensor("out", [BT, C], F16, kind="ExternalOutput").ap()

    with tile.TileContext(nc) as tc:
        for _ in range(repeat):
            _emit(tc, kt, vv, wqk, bqk, wv, bv, xt, wp, out)
    nc.finalize()
    return nc


def _emit(tc, kt, vv, wqk, bqk, wv, bv, xt, wp, out):
    from contextlib import ExitStack

    nc = tc.nc
    scale = 1.0 / math.sqrt(D)

    with ExitStack() as ctx:
        const = ctx.enter_context(tc.tile_pool(name="const", bufs=1))
        kv = ctx.enter_context(tc.tile_pool(name="kv", bufs=5))
        ptp = ctx.enter_context(tc.tile_pool(name="ptp", bufs=3))
        small = ctx.enter_context(tc.tile_pool(name="small", bufs=3))
        outsb = ctx.enter_context(tc.tile_pool(name="outsb", bufs=2))
        ps_big = ctx.enter_context(tc.tile_pool(name="ps_big", bufs=3, space="PSUM"))
        ps_ya = ctx.enter_context(tc.tile_pool(name="ps_ya", bufs=2, space="PSUM"))
        ps_yt = ctx.enter_context(tc.tile_pool(name="ps_yt", bufs=2, space="PSUM"))
        ps_sums = ctx.enter_context(tc.tile_pool(name="ps_sums", bufs=1, space="PSUM"))

        # ---- constants ----
        wqk_sb = const.tile([128, KC, 512], F16)
        nc.sync.dma_start(wqk_sb[:], wqk)
        wv_sb = const.tile([128, KC, 256], F16)
        nc.sync.dma_start(wv_sb[:], wv)
        xt_sb = const.tile([128, KC, 128], F16)
        nc.sync.dma_start(xt_sb[:], xt)
        bqk_sb = const.tile([128, 4], F32)
        nc.sync.dma_start(bqk_sb[:], bqk)
        bv_sb = const.tile([1, 256], F16)
        nc.sync.dma_start(bv_sb[:], bv)
        wp_sb = const.tile([128, HPC, C], F16)
        nc.sync.dma_start(wp_sb[:], wp)
        ones1 = const.tile([1, 128], F16)
        nc.vector.memset(ones1[:], 1.0)
        ones128 = const.tile([128, 1], F32)
        nc.vector.memset(ones128[:], 1.0)
        id8 = const.tile([8, 8], F32)
        make_identity(nc, id8[:])

        # ---- phase 1: projections ----
        # qkT[m] = (w_qk[:, m-block]).T @ x.T   -> [d, b*t], m in {q0,q1,k0,k1}
        qkT_sb = const.tile([128, 4 * 128], F16)
        for m in range(4):
            ps = ps_big.tile([128, 512], F32, tag="ps_big")
            for c in range(KC):
                nc.tensor.matmul(
                    ps[:, 0:128],
                    lhsT=wqk_sb[:, c, m * 128 : (m + 1) * 128],
                    rhs=xt_sb[:, c, :],
                    start=(c == 0),
                    stop=(c == KC - 1),
                )
            nc.scalar.activation(
                qkT_sb[:, m * 128 : (m + 1) * 128],
                ps[:, 0:128],
                func=mybir.ActivationFunctionType.Identity,
                bias=bqk_sb[:, m : m + 1],
                scale=1.0,
            )

        # v_new = x @ w_v + b_v  -> [b*t, hl*128+d]
        vproj_sb = const.tile([128, 256], F16)
        psv = ps_big.tile([128, 512], F32, tag="ps_big")
        for c in range(KC):
            nc.tensor.matmul(
                psv[:, 0:256],
                lhsT=xt_sb[:, c, :],
                rhs=wv_sb[:, c, :],
                start=(c == 0),
                stop=False,
            )
        nc.tensor.matmul(
            psv[:, 0:256], lhsT=ones1[:], rhs=bv_sb[:], start=False, stop=True
        )
        nc.scalar.copy(vproj_sb[:], psv[:, 0:256])

        # ---- phase 2: attention over 32 (b, hl) pairs ----
        yall_sb = const.tile([128, HPC * 128], F16)
        for b in range(B):
            for hl in range(HPC):
                kt_t = kv.tile([128, S], F16, tag="kt")
                for q in range(4):
                    hi = min((q + 1) * 1024, START_POS)
                    nc.sync.dma_start(
                        kt_t[:, q * 1024 : hi],
                        kt[b, hl, :, q * 1024 : hi],
                    )
                # overwrite the 8 new k positions (cols start_pos..start_pos+8)
                nc.vector.tensor_copy(
                    kt_t[:, START_POS : START_POS + 8],
                    qkT_sb[:, (2 + hl) * 128 + b * 8 : (2 + hl) * 128 + b * 8 + 8],
                )
                v_t = kv.tile([128, NCH, 128], F16, tag="v")
                for q in range(4):
                    nc.sync.dma_start(
                        v_t[:, q * 8 : (q + 1) * 8, :], vv[b, hl, :, q * 8 : (q + 1) * 8, :]
                    )
                # overwrite the 8 new v rows (chunk 31, partitions 120..128)
                nc.gpsimd.dma_start(
                    v_t[120:128, NCH - 1, 0:128],
                    vproj_sb[b * 8 : (b + 1) * 8, hl * 128 : (hl + 1) * 128],
                )

                qT_b = qkT_sb[:, hl * 128 + b * 8 : hl * 128 + b * 8 + 8]

                # scoresT: [s % 128, chunk*8 + t]
                ps_s = ps_big.tile([128, 512], F32, tag="ps_big")
                for c in range(NCH):
                    nc.tensor.matmul(
                        ps_s[:, c * 8 : (c + 1) * 8],
                        lhsT=kt_t[:, c * 128 : (c + 1) * 128],
                        rhs=qT_b,
                        start=(c == 0),
                        stop=(c == NCH - 1),
                    )
                pt_sb = ptp.tile([128, 256], F16)
                nc.scalar.activation(
                    pt_sb[:],
                    ps_s[:, 0:256],
                    func=mybir.ActivationFunctionType.Exp,
                    scale=scale,
                )
                # y = p.T @ v : [t, d]
                ya = ps_ya.tile([8, 128], F32)
                for c in range(NCH):
                    nc.tensor.matmul(
                        ya[:],
                        lhsT=pt_sb[:, c * 8 : (c + 1) * 8],
                        rhs=v_t[:, c, :],
                        start=(c == 0),
                        stop=(c == NCH - 1),
                    )
                # softmax sums: reduce p over chunks on DVE, then over s%128 on PE
                csum = small.tile([128, 8], F32, tag="csum")
                nc.vector.reduce_sum(
                    csum[:], pt_sb[:].rearrange("p (c t) -> p t c", t=8),
                    axis=mybir.AxisListType.X,
                )
                sums = ps_sums.tile([8, 1], F32, tag="sums")
                nc.tensor.matmul(sums[:], lhsT=csum[:], rhs=ones128[:])
                rec = small.tile([8, 1], F32, tag="rec")
                nc.vector.reciprocal(rec[:], sums[:])
                yn = small.tile([8, 128], F32, tag="yn")
                nc.vector.tensor_scalar_mul(yn[:], ya[:], rec[:])
                # transpose to [d, t] and park in yall
                yt = ps_yt.tile([128, 8], F32)
                nc.tensor.transpose(yt[:], yn[:], id8[:])
                nc.vector.tensor_copy(
                    yall_sb[:, hl * 128 + b * 8 : hl * 128 + b * 8 + 8], yt[:]
                )

        # ---- phase 3: partial out-projection (fp32) ----
        for nb in range(4):
            pso = ps_big.tile([128, 512], F32, tag="ps_big")
            for kc in range(HPC):
                nc.tensor.matmul(
                    pso[:],
                    lhsT=yall_sb[:, kc * 128 : (kc + 1) * 128],
                    rhs=wp_sb[:, kc, nb * 512 : (nb + 1) * 512],
                    start=(kc == 0),
                    stop=(kc == HPC - 1),
                )
            osb = outsb.tile([128, 512], F16)
            nc.vector.tensor_copy(osb[:], pso[:])
            nc.sync.dma_start(out[:, nb * 512 : (nb + 1) * 512], osb[:])


def _prep_core_inputs(core, x2d, k_cache, v_cache, w_attn, b_attn, w_proj):
    hg0 = HPC * core
    f16 = np.float16

    # wqk[p, c, m*128+j]: m in {q_h0, q_h1, k_h0, k_h1}
    cols = []
    for m in range(2):  # q block then k block
        for hl in range(HPC):
            base = m * C + (hg0 + hl) * D
            cols.append(np.arange(base, base + D))
    cols = np.concatenate(cols)  # [512]
    wqk = np.ascontiguousarray(
        w_attn[:, cols].reshape(KC, 128, 512).transpose(1, 0, 2)
    ).astype(f16)
    bqk = np.ascontiguousarray(b_attn[cols].reshape(4, 128).T).astype(np.float32)

    vcols = np.arange(2 * C + hg0 * D, 2 * C + (hg0 + HPC) * D)  # [256]
    wv = np.ascontiguousarray(
        w_attn[:, vcols].reshape(KC, 128, 256).transpose(1, 0, 2)
    ).astype(f16)
    bv = b_attn[vcols].reshape(1, 256).astype(f16)

    xt = np.ascontiguousarray(x2d.T.reshape(KC, 128, 128).transpose(1, 0, 2)).astype(
        f16
    )

    wpl = w_proj[hg0 * D : (hg0 + HPC) * D, :]  # [256, C]
    wp = np.ascontiguousarray(wpl.reshape(HPC, 128, C).transpose(1, 0, 2)).astype(
        np.float16
    )

    return {
        "wqk": wqk,
        "bqk": bqk,
        "wv": wv,
        "bv": bv,
        "xt": xt,
        "wp": wp,
    }


def _prep_big_concat(k_cache, v_cache):
    """Build the concatenated kt/vv arrays for all 8 cores in one fused
    strided-cast pass per core chunk (threaded)."""
    from concurrent.futures import ThreadPoolExecutor

    f16 = np.float16
    kt_cat = np.empty((N_CORES * B, HPC, D, S), f16)
    vv_cat = np.empty((N_CORES * B, HPC, 128, NCH, 128), f16)

    def fill(core):
        hg0 = HPC * core
        sl = slice(core * B, (core + 1) * B)
        kt_cat[sl] = k_cache[:, hg0 : hg0 + HPC].transpose(0, 1, 3, 2)
        vv_cat[sl] = (
            v_cache[:, hg0 : hg0 + HPC]
            .reshape(B, HPC, NCH, 128, D)
            .transpose(0, 1, 3, 2, 4)
        )

    with ThreadPoolExecutor(max_workers=8) as ex:
        list(ex.map(fill, range(N_CORES)))
    return kt_cat, vv_cat


def _get_runner():
    """Compile once per process: returns (sharded_jit, in_names, mesh_sharding)."""
    if "runner" in _CACHE:
        return _CACHE["runner"]
    import jax
    from jax.sharding import Mesh, NamedSharding, PartitionSpec
    from jax.experimental.shard_map import shard_map
    from concourse import bass2jax

    nc = _build_nc()
    bass2jax.install_neuronx_cc_hook()
    partition_name = nc.partition_id_tensor.name if nc.partition_id_tensor else None

    in_names, out_names, out_avals, zero_outs = [], [], [], []
    for alloc in nc.m.functions[0].allocations:
        if not isinstance(alloc, mybir.MemoryLocationSet):
            continue
        name = alloc.memorylocations[0].name
        if alloc.kind == "ExternalInput":
            if name != partition_name:
                in_names.append(name)
        elif alloc.kind == "ExternalOutput":
            out_names.append(name)
            shape = tuple(alloc.tensor_shape)
            dtype = mybir.dt.np(alloc.dtype)
            out_avals.append(jax.core.ShapedArray(shape, dtype))
            zero_outs.append(np.zeros(shape, dtype))
    n_params = len(in_names)
    all_in_names = list(in_names) + list(out_names)
    if partition_name is not None:
        all_in_names.append(partition_name)

    def _body(*args):
        operands = list(args)
        if partition_name is not None:
            operands.append(bass2jax.partition_id_tensor())
        outs = bass2jax._bass_exec_p.bind(
            *operands,
            out_avals=tuple(out_avals),
            in_names=tuple(all_in_names),
            out_names=tuple(out_names),
            lowering_input_output_aliases=(),
            sim_require_finite=True,
            sim_require_nnan=True,
            nc=nc,
        )
        return tuple(outs)

    devices = jax.devices()[:N_CORES]
    mesh = Mesh(np.asarray(devices), ("core",))
    in_specs = (PartitionSpec("core"),) * (n_params + len(out_names))
    out_specs = (PartitionSpec("core"),) * len(out_names)
    sharded = jax.jit(
        shard_map(_body, mesh=mesh, in_specs=in_specs, out_specs=out_specs,
                  check_rep=False),
        keep_unused=True,
    )
    sh = NamedSharding(mesh, PartitionSpec("core"))
    dev_zeros = [
        jax.device_put(np.zeros((N_CORES * z.shape[0], *z.shape[1:]), z.dtype), sh)
        for z in zero_outs
    ]
    _CACHE["runner"] = (sharded, in_names, out_names, out_avals, sh, dev_zeros)
    return _CACHE["runner"]


def kernel(
    x,
    k_cache,
    v_cache,
    w_attn,
    b_attn,
    w_proj,
    b_proj,
    start_pos,
    is_causal,
):
    x = np.asarray(x, dtype=np.float32)
    k_cache = np.asarray(k_cache, dtype=np.float32)
    v_cache = np.asarray(v_cache, dtype=np.float32)
    w_attn = np.asarray(w_attn, dtype=np.float32)
    b_attn = np.asarray(b_attn, dtype=np.float32)
    w_proj = np.asarray(w_proj, dtype=np.float32)
    b_proj = np.asarray(b_proj, dtype=np.float32)
    assert int(start_pos) == START_POS, f"kernel hardcodes start_pos={START_POS}"
    assert int(is_causal) == 0, "kernel hardcodes is_causal=0"

    sharded, in_names, out_names, out_avals, sh, dev_zeros = _get_runner()

    x2d = x.reshape(BT, C)
    kt_cat, vv_cat = _prep_big_concat(k_cache, v_cache)
    in_maps = [
        _prep_core_inputs(c, x2d, k_cache, v_cache, w_attn, b_attn, w_proj)
        for c in range(N_CORES)
    ]
    big = {"kt": kt_cat, "vv": vv_cat}
    concat_in = [
        big[nm]
        if nm in big
        else np.concatenate([in_maps[c][nm] for c in range(N_CORES)], axis=0)
        for nm in in_names
    ]
    outs = sharded(*concat_in, *dev_zeros)
    partial = (
        np.asarray(outs[0])
        .astype(np.float64)
        .reshape(N_CORES, BT, C)
        .sum(axis=0)
    )
    y = (partial + b_proj).astype(np.float32)
    return y.reshape(B, T, C)


if __name__ == "__main__":
    # quick self-run against the local reference
    sys.path.insert(0, "/root/problem")
    import reference

    inputs = {k: np.asarray(v) for k, v in reference.setup_inputs().items()}
    expected = np.asarray(reference.reference(**reference.setup_inputs()))
    actual = kernel(**inputs)
    err = np.abs(actual - expected)
    rel = err.max() / np.abs(expected).max()
    print("max abs err:", err.max(), "rel:", rel)
